# revision 7
# baseline (speedup 1.0000x reference)
"""MixedExpertLayer Trainium2 kernel, v13: device does MLP experts only.

ALL four experts are routed on the host. For the two MLP experts the tokens
with nonzero combined weight c_e = sum_k w_k*[idx_k==e] (~43.75% of tokens)
are gathered into a compacted feature-major stream and split EVENLY across
the 8 cores, so each core processes ~900 tokens per expert.

MLP experts 0,1 (per core, per expert, windows of <=512 tokens):
  gate/up: psum[i-tile, tok] = sum_h W[h,i]^T x[h, tok]   (feature-major)
  A = silu(g)*u on ACT+DVE
  down (feature-major): psum[h-tile, tok] = sum_i Wd[i,h]^T A[i, tok]
  scaled by c_e via one DVE mul with a host-broadcast coefficient row.

Conv experts 2,3 (depthwise causal conv1d + silu) are computed entirely on
the host: the routed gather already required materializing the tap-shifted
token copies host-side, which costs more than the conv itself, and keeping
them off the device removes ~15MB/core of DMA plus the DVE/ACT drain work
that stalled the PE between matmul groups.

DMA layout is tuned around the ~0.6us serial HWDGE descriptor-gen cost per
dma_start: activations are packed per-(expert,window) contiguous (1 desc
each), weights in 4-i-tile 1MB chunks, the first gate/up tiles go on the
ACT HWDGE queue (runs in parallel with the SP queue), and output stores go
on ACT so they never contend with the SP weight stream; the two final
stores are split across SP and ACT so their descriptor-gens overlap.

All outputs are compacted feature-major [H, C_e]; the host transposes and
scatter-adds the streams into the zero-initialized result (fp32).
"""

import numpy as np
import ml_dtypes

import concourse.bass as bass
import concourse.mybir as mybir
import concourse.tile as tile
from concourse.bass_utils import run_bass_kernel_spmd

B, S, H, I, KTOP, KC = 4, 4096, 1024, 2048, 2, 4
NCORES = 8
TCH = 512                      # token chunk (matmul N / PSUM bank)
HK = H // 128                  # 8 h-chunks
IK = I // 128                  # 16 i-chunks
NQ = 4                         # i-tile chunks per expert (IK/4)
NWARM = 9                      # PE clock-warmup matmuls
BF16 = mybir.dt.bfloat16
F32 = mybir.dt.float32
AF = mybir.ActivationFunctionType

# routing state set by build_in_maps: per-expert device capacities and
# per-(expert, core) global token lists
_ROUTE = {"C": [1024, 1024], "lists": None}


def legalize_waits(nc):
    """This walrus build encodes exactly one sync-wait per instruction
    (single NEURON_ISA_TPB_EVENTS slot); Tile emits up to 3 plus a multi-wait
    tail Drain. Split extra waits onto wait-only EventSemaphore carriers
    inserted immediately before the instruction (same engine, same position,
    so no reordering and no deadlock risk)."""
    f = nc.m.functions[0]
    for blk in f.blocks:
        new = []
        for ins in list(blk.instructions):
            si = ins.sync_info
            if si is not None and si.on_wait and len(si.on_wait) > 1:
                best, order = {}, []
                for w in si.on_wait:
                    k = (w.sync_type, w.id, w.wait_mode)
                    if k not in best:
                        best[k] = w
                        order.append(k)
                    elif (w.wait_value or 0) > (best[k].wait_value or 0):
                        best[k] = w
                waits = [best[k] for k in order]
                for j, w in enumerate(waits[:-1]):
                    ev = mybir.InstEventSemaphore(
                        name=f"{ins.name}-lw{j}", engine=ins.engine, ins=[], outs=[],
                    )
                    ev.sync_info = mybir.SyncInfo(on_wait=[w], on_update=[])
                    new.append(ev)
                si.on_wait = [waits[-1]]
                ins.sync_info = si
            new.append(ins)
        blk.instructions = new
    return nc


def _windows(Ce):
    """Chunk windows (w0, n) covering Ce tokens in <=TCH pieces."""
    w, out = 0, []
    while w < Ce:
        n = min(TCH, Ce - w)
        out.append((w, n))
        w += n
    return out


def build_nc():
    C0, C1 = _ROUTE["C"][:2]
    nc = bass.Bass(num_devices=NCORES)
    # activations packed per-(expert, window): [128, sum_w HK*nw]
    xg0 = nc.dram_tensor("xg0", [128, HK * C0], BF16, kind="ExternalInput")
    xg1 = nc.dram_tensor("xg1", [128, HK * C1], BF16, kind="ExternalInput")
    # weights in 4-i-tile chunks: [e, q, part, j, hk, 128]
    wgr = nc.dram_tensor("wgr", [2, NQ, 128, 4, HK, 128], BF16, kind="ExternalInput")
    wur = nc.dram_tensor("wur", [2, NQ, 128, 4, HK, 128], BF16, kind="ExternalInput")
    wdr = nc.dram_tensor("wdr", [2, NQ, 128, 4, HK, 128], BF16, kind="ExternalInput")
    cgb0 = nc.dram_tensor("cgb0", [128, C0], BF16, kind="ExternalInput")
    cgb1 = nc.dram_tensor("cgb1", [128, C1], BF16, kind="ExternalInput")
    yf0 = nc.dram_tensor("yf0", [H, C0], BF16, kind="ExternalOutput")
    yf1 = nc.dram_tensor("yf1", [H, C1], BF16, kind="ExternalOutput")

    yf_t = [y.rearrange("(o p) t -> p o t", p=128) for y in (yf0, yf1)]
    xg_d = [xg0, xg1]
    cgb_d = [cgb0, cgb1]
    wins = [_windows(C0), _windows(C1)]
    phases = [(0, wi, w0, n) for wi, (w0, n) in enumerate(wins[0])] + \
             [(1, wi, w0, n) for wi, (w0, n) in enumerate(wins[1])]

    with tile.TileContext(nc) as tc:
        with (
            tc.tile_pool(name="singles", bufs=1) as singles,
            tc.tile_pool(name="wpool", bufs=4) as wpool,
            tc.tile_pool(name="wdpool", bufs=4) as wdpool,
            tc.tile_pool(name="apool", bufs=2) as apool,
            tc.tile_pool(name="sgp", bufs=2) as sgpool,
            tc.tile_pool(name="ytp", bufs=3) as ytpool,
            tc.tile_pool(name="psg", bufs=3, space="PSUM") as psgp,
            tc.tile_pool(name="psu", bufs=3, space="PSUM") as psup,
            tc.tile_pool(name="pd", bufs=2, space="PSUM") as pd,
        ):
            # ---- PE clock warmup: dummy matmuls with no DMA deps run while
            # the first input tiles stream in, so the HAM clock-gate ramp
            # (1.2 -> 2.4GHz after ~3.4-5us of sustained PE activity)
            # finishes about when the real matmuls start ----
            scr = singles.tile([128, TCH], BF16)
            nc.gpsimd.memset(scr, 0.0)
            pw = pd.tile([128, TCH], F32, tag="pd", name="pwarm")
            for r in range(NWARM):
                nc.tensor.matmul(pw, scr[:, 0:128], scr,
                                 start=(r == 0), stop=(r == NWARM - 1))

            # ---- critical head DMAs: the ACT HWDGE queue carries the first
            # gate/up i-tile (its desc-gen runs in parallel with SP's), SP
            # carries the first activation window ----
            xgw_sb = [[singles.tile([128, HK * n], BF16, name=f"xg{e}w{wi}")
                       for wi, (w0, n) in enumerate(wins[e])]
                      for e in range(2)]
            cgb_sb = [singles.tile([128, Cx], BF16, name=f"cgb{i}")
                      for i, Cx in enumerate((C0, C1))]

            def xg_load(e, wi):
                off = HK * wins[e][wi][0]
                n = wins[e][wi][1]
                nc.sync.dma_start(xgw_sb[e][wi], xg_d[e][:, off : off + HK * n])

            wg_t = {}   # (e, q) -> sbuf chunk tile [128, 4, HK, 128]
            wu_t = {}
            wd_t = {}

            def wgu_alloc(e, q):
                wg_t[(e, q)] = wpool.tile([128, 4, HK, 128], BF16, tag="wg",
                                          name="wgt")
                wu_t[(e, q)] = wpool.tile([128, 4, HK, 128], BF16, tag="wu",
                                          name="wut")

            def wgu_load(e, q):
                nc.sync.dma_start(wg_t[(e, q)], wgr[e, q])
                nc.sync.dma_start(wu_t[(e, q)], wur[e, q])

            def wd_load(e, q):
                wd_t[(e, q)] = wdpool.tile([128, 4, HK, 128], BF16, tag="wd",
                                           name="wdt")
                nc.sync.dma_start(wd_t[(e, q)], wdr[e, q])

            # head: everything the first matmul groups need goes on the SP
            # HWDGE ring IN DEMAND ORDER - the ring drains FIFO per issuing
            # engine, so issue order IS completion order.  Concurrent DMAs
            # on other rings would steal bandwidth via packet round-robin,
            # so only late-needed small loads ride the ACT ring.
            wgu_alloc(0, 0)
            nc.sync.dma_start(wg_t[(0, 0)][:, 0], wgr[0, 0, :, 0])
            nc.sync.dma_start(wu_t[(0, 0)][:, 0], wur[0, 0, :, 0])
            n0 = wins[0][0][1]
            h2 = HK // 2
            nc.sync.dma_start(xgw_sb[0][0][:, 0 : h2 * n0],
                              xg_d[0][:, 0 : h2 * n0])
            nc.sync.dma_start(xgw_sb[0][0][:, h2 * n0 : HK * n0],
                              xg_d[0][:, h2 * n0 : HK * n0])
            nc.sync.dma_start(wg_t[(0, 0)][:, 1:4], wgr[0, 0, :, 1:4])
            nc.sync.dma_start(wu_t[(0, 0)][:, 1:4], wur[0, 0, :, 1:4])
            # late-needed small loads on the ACT ring (desc-gens run before
            # the first silu reaches the ACT queue)
            nc.scalar.dma_start(cgb_sb[0], cgb_d[0][:])
            nc.scalar.dma_start(cgb_sb[1], cgb_d[1][:])
            off01 = HK * wins[0][1][0]
            n01 = wins[0][1][1]
            nc.scalar.dma_start(xgw_sb[0][1],
                                xg_d[0][:, off01 : off01 + HK * n01])

            # deferred SP loads, emitted at (q, j) points of expert 0's first
            # window in the order the data is needed
            def defer_e0(q, j):
                if (q, j) == (0, 1):
                    wgu_alloc(0, 1); wgu_load(0, 1)
                elif (q, j) == (0, 3):
                    wgu_alloc(0, 2); wgu_load(0, 2)
                elif (q, j) == (1, 0):
                    xg_load(1, 0)
                elif (q, j) == (1, 1):
                    xg_load(1, 1)
                elif (q, j) == (1, 2):
                    wgu_alloc(0, 3); wgu_load(0, 3)
                elif (q, j) == (2, 0):
                    wd_load(0, 0)
                elif (q, j) == (2, 2):
                    wd_load(0, 1)
                elif (q, j) == (3, 0):
                    wd_load(0, 2)
                elif (q, j) == (3, 2):
                    wd_load(0, 3)

            def defer_e1(q, j):
                if j == 0:
                    wgu_alloc(1, q); wgu_load(1, q)
                elif q >= 2 and j >= 2:
                    wd_load(1, 2 * (q - 2) + (j - 2))

            for pi, (e, wi, w0, nw) in enumerate(phases):
                xgw = xgw_sb[e][wi]
                wload = (wi == 0)
                # ---- gate/up -> A (feature-major [I, nw]) ----
                a_sb = apool.tile([128, IK, TCH], BF16, tag="a")
                for q in range(NQ):
                    for j in range(4):
                        if wload and not (pi == 0 and q == 0 and j == 0):
                            (defer_e0 if e == 0 else defer_e1)(q, j)
                        i = 4 * q + j
                        psg = psgp.tile([128, TCH], F32, tag="pg")
                        psu = psup.tile([128, TCH], F32, tag="pu")
                        for kc in range(HK):
                            nc.tensor.matmul(
                                psg[:, 0:nw], wg_t[(e, q)][:, j, kc, :],
                                xgw[:, kc * nw : (kc + 1) * nw],
                                start=(kc == 0), stop=(kc == HK - 1))
                        for kc in range(HK):
                            nc.tensor.matmul(
                                psu[:, 0:nw], wu_t[(e, q)][:, j, kc, :],
                                xgw[:, kc * nw : (kc + 1) * nw],
                                start=(kc == 0), stop=(kc == HK - 1))
                        sg = sgpool.tile([128, TCH], F32, tag="sg")
                        nc.scalar.activation(
                            out=sg[:, 0:nw], in_=psg[:, 0:nw], func=AF.Silu)
                        nc.vector.tensor_mul(
                            a_sb[:, i, 0:nw], sg[:, 0:nw], psu[:, 0:nw])

                # ---- down, feature-major: psum[h-tile, tok] ----
                # the very last group is split by columns so the final
                # mul+store chain after the last matmul is half-sized, and
                # its two store descriptor-gens run in parallel on SP + ACT
                last_pi = pi == len(phases) - 1
                for hb in range(HK):
                    split = last_pi and hb == HK - 1
                    cols = [(0, nw // 2), (nw // 2, nw)] if split \
                        else [(0, nw)]
                    for si, (c0, c1) in enumerate(cols):
                        psd = pd.tile([128, TCH], F32, tag="pd")
                        for kc in range(IK):
                            q, j = divmod(kc, 4)
                            nc.tensor.matmul(
                                psd[:, 0 : c1 - c0], wd_t[(e, q)][:, j, hb, :],
                                a_sb[:, kc, c0:c1],
                                start=(kc == 0), stop=(kc == IK - 1))
                        yt = ytpool.tile([128, TCH], BF16, tag="yt")
                        nc.vector.tensor_mul(
                            yt[:, 0 : c1 - c0], psd[:, 0 : c1 - c0],
                            cgb_sb[e][:, w0 + c0 : w0 + c1])
                        # stores ride the ACT HWDGE queue (idle during the
                        # down phase) so they never contend with SP's weight
                        # stream; the very last store goes on SP
                        eng = nc.sync if (split and si == 1) else nc.scalar
                        eng.dma_start(
                            yf_t[e][:, hb, w0 + c0 : w0 + c1],
                            yt[:, 0 : c1 - c0])
    return legalize_waits(nc)


def _bf16(a):
    return np.asarray(a).astype(ml_dtypes.bfloat16)


def _chunk4(w):
    """[2, IK, 128, HK, 128] -> [2, NQ, 128, 4, HK, 128] contiguous."""
    return np.ascontiguousarray(
        w.reshape(2, NQ, 4, 128, HK, 128).transpose(0, 1, 3, 2, 4, 5))


def build_in_maps(x, top_k_indices, norm_weights, mlp_gate, mlp_up, mlp_down, conv_w):
    NT = B * S
    xflat = np.asarray(x, dtype=np.float32).reshape(NT, H)
    xflat_b = _bf16(xflat)
    idxflat = np.asarray(top_k_indices).reshape(NT, KTOP)
    nwflat = np.asarray(norm_weights, dtype=np.float32).reshape(NT, KTOP)

    # combined per-expert coefficients, global
    ce = np.zeros((NT, 4), dtype=np.float32)
    rows = np.arange(NT)
    for k in range(KTOP):
        np.add.at(ce, (rows, idxflat[:, k]), nwflat[:, k])

    # globally balanced routing for the two MLP experts: split every
    # expert's token list evenly across the cores
    lists, Cs = [], []
    for e in range(2):
        glst = np.nonzero(ce[:, e] != 0.0)[0]
        parts = np.array_split(glst, NCORES)
        lists.append(parts)
        Cs.append(max(1, max(len(p) for p in parts)))
    _ROUTE["C"] = Cs
    _ROUTE["lists"] = lists

    # conv experts 2,3 computed fully on the host (see module docstring):
    # depthwise causal conv + silu, weighted by c_e, over each expert's
    # routed tokens only
    cwf = np.asarray(conv_w, dtype=np.float32)            # [2, H, KC]
    conv_add = np.zeros((NT, H), dtype=np.float32)
    for e in range(2):
        glst = np.nonzero(ce[:, 2 + e] != 0.0)[0]
        s_in_seq = glst % S
        z = np.zeros((len(glst), H), dtype=np.float32)
        for j in range(KC):
            src = glst + j - (KC - 1)
            valid = (s_in_seq + j - (KC - 1)) >= 0
            z += np.where(valid[:, None], xflat[src * valid], 0) \
                * cwf[e, :, j][None, :]
        z = z / (1.0 + np.exp(-z))
        conv_add[glst] += z * ce[glst, 2 + e][:, None]
    _ROUTE["conv_add"] = conv_add

    # weights, repacked into 4-i-tile chunks contiguous per partition
    wgr = _chunk4(
        _bf16(mlp_gate).reshape(2, HK, 128, IK, 128).transpose(0, 3, 2, 1, 4))
    wur = _chunk4(
        _bf16(mlp_up).reshape(2, HK, 128, IK, 128).transpose(0, 3, 2, 1, 4))
    wdr = _chunk4(_bf16(mlp_down).reshape(2, IK, 128, HK, 128))

    def fm_pack(cols_bf16, Cx):
        """[n, H] bf16 -> [128, HK*Cx] zero-padded feature-major, packed
        per-window so each window is one contiguous DMA."""
        n = cols_bf16.shape[0]
        arr = np.zeros((H, Cx), dtype=ml_dtypes.bfloat16)
        arr[:, :n] = cols_bf16.T
        a3 = arr.reshape(HK, 128, Cx)
        parts = [
            a3[:, :, w0 : w0 + nw].transpose(1, 0, 2).reshape(128, HK * nw)
            for (w0, nw) in _windows(Cx)
        ]
        return np.ascontiguousarray(np.concatenate(parts, axis=1))

    def bcast_row(vals, Cx):
        v = np.zeros(Cx, dtype=np.float32)
        v[: len(vals)] = vals
        return np.ascontiguousarray(
            np.broadcast_to(v[None, :], (128, Cx))).astype(ml_dtypes.bfloat16)

    in_maps = []
    for i in range(NCORES):
        im = {"wgr": wgr, "wur": wur, "wdr": wdr}
        for e in range(2):
            lst = lists[e][i]
            im[f"xg{e}"] = fm_pack(xflat_b[lst], Cs[e])
            im[f"cgb{e}"] = bcast_row(ce[lst, e], Cs[e])
        in_maps.append(im)
    return in_maps


def assemble(results):
    lists = _ROUTE["lists"]
    out = _ROUTE["conv_add"].copy()
    keys = ["yf0", "yf1"]
    for i, r in enumerate(results):
        for e in range(2):
            lst = lists[e][i]
            n = len(lst)
            yv = np.asarray(r[keys[e]], dtype=np.float32)  # [H, C_e]
            out[lst] += yv[:, :n].T
    return out.reshape(B, S, H)


def kernel(x, top_k_indices, norm_weights, mlp_gate, mlp_up, mlp_down, conv_w):
    in_maps = build_in_maps(
        x, top_k_indices, norm_weights, mlp_gate, mlp_up, mlp_down, conv_w
    )
    nc = build_nc()
    res = run_bass_kernel_spmd(nc, in_maps, core_ids=list(range(NCORES)))
    return assemble(res.results)


# revision 9
# speedup vs baseline: 1.0194x; 1.0194x over previous
"""MixedExpertLayer Trainium2 kernel, v13: device does MLP experts only.

ALL four experts are routed on the host. For the two MLP experts the tokens
with nonzero combined weight c_e = sum_k w_k*[idx_k==e] (~43.75% of tokens)
are gathered into a compacted feature-major stream and split EVENLY across
the 8 cores, so each core processes ~900 tokens per expert.

MLP experts 0,1 (per core, per expert, windows of <=512 tokens):
  gate/up: psum[i-tile, tok] = sum_h W[h,i]^T x[h, tok]   (feature-major)
  A = silu(g)*u on ACT+DVE
  down (feature-major): psum[h-tile, tok] = sum_i Wd[i,h]^T A[i, tok]
  scaled by c_e via one DVE mul with a host-broadcast coefficient row.

Conv experts 2,3 (depthwise causal conv1d + silu) are computed entirely on
the host: the routed gather already required materializing the tap-shifted
token copies host-side, which costs more than the conv itself, and keeping
them off the device removes ~15MB/core of DMA plus the DVE/ACT drain work
that stalled the PE between matmul groups.

DMA layout is tuned around the ~0.6us serial HWDGE descriptor-gen cost per
dma_start: activations are packed per-(expert,window) contiguous (1 desc
each), weights in 4-i-tile 1MB chunks, the first gate/up tiles go on the
ACT HWDGE queue (runs in parallel with the SP queue), and output stores go
on ACT so they never contend with the SP weight stream; the two final
stores are split across SP and ACT so their descriptor-gens overlap.

All outputs are compacted feature-major [H, C_e]; the host transposes and
scatter-adds the streams into the zero-initialized result (fp32).
"""

import numpy as np
import ml_dtypes

import concourse.bass as bass
import concourse.mybir as mybir
import concourse.tile as tile
from concourse.bass_utils import run_bass_kernel_spmd

B, S, H, I, KTOP, KC = 4, 4096, 1024, 2048, 2, 4
NCORES = 8
TCH = 512                      # token chunk (matmul N / PSUM bank)
HK = H // 128                  # 8 h-chunks
IK = I // 128                  # 16 i-chunks
NQ = 4                         # i-tile chunks per expert (IK/4)
NWARM = 14                     # PE clock-warmup matmuls
BF16 = mybir.dt.bfloat16
F32 = mybir.dt.float32
AF = mybir.ActivationFunctionType

# routing state set by build_in_maps: per-expert device capacities and
# per-(expert, core) global token lists
_ROUTE = {"C": [1024, 1024], "lists": None}


def legalize_waits(nc):
    """This walrus build encodes exactly one sync-wait per instruction
    (single NEURON_ISA_TPB_EVENTS slot); Tile emits up to 3 plus a multi-wait
    tail Drain. Split extra waits onto wait-only EventSemaphore carriers
    inserted immediately before the instruction (same engine, same position,
    so no reordering and no deadlock risk)."""
    f = nc.m.functions[0]
    for blk in f.blocks:
        new = []
        for ins in list(blk.instructions):
            si = ins.sync_info
            if si is not None and si.on_wait and len(si.on_wait) > 1:
                best, order = {}, []
                for w in si.on_wait:
                    k = (w.sync_type, w.id, w.wait_mode)
                    if k not in best:
                        best[k] = w
                        order.append(k)
                    elif (w.wait_value or 0) > (best[k].wait_value or 0):
                        best[k] = w
                waits = [best[k] for k in order]
                for j, w in enumerate(waits[:-1]):
                    ev = mybir.InstEventSemaphore(
                        name=f"{ins.name}-lw{j}", engine=ins.engine, ins=[], outs=[],
                    )
                    ev.sync_info = mybir.SyncInfo(on_wait=[w], on_update=[])
                    new.append(ev)
                si.on_wait = [waits[-1]]
                ins.sync_info = si
            new.append(ins)
        blk.instructions = new
    return nc


def _windows(Ce):
    """Chunk windows (w0, n) covering Ce tokens in <=TCH pieces."""
    w, out = 0, []
    while w < Ce:
        n = min(TCH, Ce - w)
        out.append((w, n))
        w += n
    return out


def build_nc():
    C0, C1 = _ROUTE["C"][:2]
    nc = bass.Bass(num_devices=NCORES)
    # activations packed per-(expert, window): [128, sum_w HK*nw]
    xg0 = nc.dram_tensor("xg0", [128, HK * C0], BF16, kind="ExternalInput")
    xg1 = nc.dram_tensor("xg1", [128, HK * C1], BF16, kind="ExternalInput")
    # weights in 4-i-tile chunks: [e, q, part, j, hk, 128]
    wgr = nc.dram_tensor("wgr", [2, NQ, 128, 4, HK, 128], BF16, kind="ExternalInput")
    wur = nc.dram_tensor("wur", [2, NQ, 128, 4, HK, 128], BF16, kind="ExternalInput")
    wdr = nc.dram_tensor("wdr", [2, NQ, 128, 4, HK, 128], BF16, kind="ExternalInput")
    cgb0 = nc.dram_tensor("cgb0", [128, C0], BF16, kind="ExternalInput")
    cgb1 = nc.dram_tensor("cgb1", [128, C1], BF16, kind="ExternalInput")
    yf0 = nc.dram_tensor("yf0", [H, C0], BF16, kind="ExternalOutput")
    yf1 = nc.dram_tensor("yf1", [H, C1], BF16, kind="ExternalOutput")

    yf_t = [y.rearrange("(o p) t -> p o t", p=128) for y in (yf0, yf1)]
    xg_d = [xg0, xg1]
    cgb_d = [cgb0, cgb1]
    wins = [_windows(C0), _windows(C1)]
    phases = [(0, wi, w0, n) for wi, (w0, n) in enumerate(wins[0])] + \
             [(1, wi, w0, n) for wi, (w0, n) in enumerate(wins[1])]

    with tile.TileContext(nc) as tc:
        with (
            tc.tile_pool(name="singles", bufs=1) as singles,
            tc.tile_pool(name="wpool", bufs=4) as wpool,
            tc.tile_pool(name="wdpool", bufs=4) as wdpool,
            tc.tile_pool(name="apool", bufs=2) as apool,
            tc.tile_pool(name="sgp", bufs=2) as sgpool,
            tc.tile_pool(name="ytp", bufs=3) as ytpool,
            tc.tile_pool(name="psg", bufs=3, space="PSUM") as psgp,
            tc.tile_pool(name="psu", bufs=3, space="PSUM") as psup,
            tc.tile_pool(name="pd", bufs=2, space="PSUM") as pd,
        ):
            # ---- PE clock warmup: dummy matmuls with no DMA deps run while
            # the first input tiles stream in, so the HAM clock-gate ramp
            # (1.2 -> 2.4GHz after ~3.4-5us of sustained PE activity)
            # finishes about when the real matmuls start ----
            scr = singles.tile([128, TCH], BF16)
            nc.gpsimd.memset(scr, 0.0)
            pw = pd.tile([128, TCH], F32, tag="pd", name="pwarm")
            for r in range(NWARM):
                nc.tensor.matmul(pw, scr[:, 0:128], scr,
                                 start=(r == 0), stop=(r == NWARM - 1))

            # ---- critical head DMAs: the ACT HWDGE queue carries the first
            # gate/up i-tile (its desc-gen runs in parallel with SP's), SP
            # carries the first activation window ----
            xgw_sb = [[singles.tile([128, HK * n], BF16, name=f"xg{e}w{wi}")
                       for wi, (w0, n) in enumerate(wins[e])]
                      for e in range(2)]
            cgb_sb = [singles.tile([128, Cx], BF16, name=f"cgb{i}")
                      for i, Cx in enumerate((C0, C1))]

            def xg_load(e, wi):
                off = HK * wins[e][wi][0]
                n = wins[e][wi][1]
                nc.sync.dma_start(xgw_sb[e][wi], xg_d[e][:, off : off + HK * n])

            wg_t = {}   # (e, q) -> sbuf chunk tile [128, 4, HK, 128]
            wu_t = {}
            wd_t = {}

            def wgu_alloc(e, q):
                wg_t[(e, q)] = wpool.tile([128, 4, HK, 128], BF16, tag="wg",
                                          name="wgt")
                wu_t[(e, q)] = wpool.tile([128, 4, HK, 128], BF16, tag="wu",
                                          name="wut")

            def wgu_load(e, q):
                nc.sync.dma_start(wg_t[(e, q)], wgr[e, q])
                nc.sync.dma_start(wu_t[(e, q)], wur[e, q])

            def wd_load(e, q):
                wd_t[(e, q)] = wdpool.tile([128, 4, HK, 128], BF16, tag="wd",
                                           name="wdt")
                nc.sync.dma_start(wd_t[(e, q)], wdr[e, q])

            # head: EVERY early load goes on the SP HWDGE ring, in demand
            # order.  Within one ring, transfers complete FIFO at full spray
            # rate once the queue is deep; a second active ring (ACT) would
            # steal ~half the bandwidth via packet round-robin, so the ACT
            # ring is reserved for the output stores (which only start once
            # the down phase begins, when the SP load queue has drained).
            wgu_alloc(0, 0)
            nc.sync.dma_start(wg_t[(0, 0)][:, 0], wgr[0, 0, :, 0])
            n0 = wins[0][0][1]
            h2 = HK // 2
            nc.sync.dma_start(xgw_sb[0][0][:, 0 : h2 * n0],
                              xg_d[0][:, 0 : h2 * n0])
            nc.sync.dma_start(xgw_sb[0][0][:, h2 * n0 : HK * n0],
                              xg_d[0][:, h2 * n0 : HK * n0])
            nc.sync.dma_start(wu_t[(0, 0)][:, 0], wur[0, 0, :, 0])
            nc.sync.dma_start(wg_t[(0, 0)][:, 1:4], wgr[0, 0, :, 1:4])
            nc.sync.dma_start(wu_t[(0, 0)][:, 1:4], wur[0, 0, :, 1:4])

            # deferred SP loads, emitted at (q, j) points of expert 0's first
            # window in the order the data is needed
            def defer_e0(q, j):
                if (q, j) == (0, 1):
                    wgu_alloc(0, 1)
                    nc.sync.dma_start(wg_t[(0, 1)], wgr[0, 1])
                elif (q, j) == (0, 2):
                    nc.sync.dma_start(wu_t[(0, 1)], wur[0, 1])
                elif (q, j) == (0, 3):
                    nc.sync.dma_start(cgb_sb[0], cgb_d[0][:])
                    nc.sync.dma_start(cgb_sb[1], cgb_d[1][:])
                elif (q, j) == (1, 0):
                    xg_load(0, 1)
                elif (q, j) == (1, 1):
                    wgu_alloc(0, 2)
                    nc.sync.dma_start(wg_t[(0, 2)], wgr[0, 2])
                elif (q, j) == (1, 2):
                    nc.sync.dma_start(wu_t[(0, 2)], wur[0, 2])
                elif (q, j) == (1, 3):
                    xg_load(1, 0)
                elif (q, j) == (2, 0):
                    xg_load(1, 1)
                elif (q, j) == (2, 1):
                    wgu_alloc(0, 3); wgu_load(0, 3)
                elif (q, j) == (2, 3):
                    wd_load(0, 0)
                elif (q, j) == (3, 0):
                    wd_load(0, 1)
                elif (q, j) == (3, 1):
                    wd_load(0, 2)
                elif (q, j) == (3, 2):
                    wd_load(0, 3)

            def defer_e1(q, j):
                if j == 0:
                    wgu_alloc(1, q); wgu_load(1, q)
                elif q >= 2 and j >= 2:
                    wd_load(1, 2 * (q - 2) + (j - 2))

            for pi, (e, wi, w0, nw) in enumerate(phases):
                xgw = xgw_sb[e][wi]
                wload = (wi == 0)
                # ---- gate/up -> A (feature-major [I, nw]) ----
                a_sb = apool.tile([128, IK, TCH], BF16, tag="a")
                for q in range(NQ):
                    for j in range(4):
                        if wload and not (pi == 0 and q == 0 and j == 0):
                            (defer_e0 if e == 0 else defer_e1)(q, j)
                        i = 4 * q + j
                        psg = psgp.tile([128, TCH], F32, tag="pg")
                        psu = psup.tile([128, TCH], F32, tag="pu")
                        for kc in range(HK):
                            nc.tensor.matmul(
                                psg[:, 0:nw], wg_t[(e, q)][:, j, kc, :],
                                xgw[:, kc * nw : (kc + 1) * nw],
                                start=(kc == 0), stop=(kc == HK - 1))
                        for kc in range(HK):
                            nc.tensor.matmul(
                                psu[:, 0:nw], wu_t[(e, q)][:, j, kc, :],
                                xgw[:, kc * nw : (kc + 1) * nw],
                                start=(kc == 0), stop=(kc == HK - 1))
                        sg = sgpool.tile([128, TCH], F32, tag="sg")
                        nc.scalar.activation(
                            out=sg[:, 0:nw], in_=psg[:, 0:nw], func=AF.Silu)
                        nc.vector.tensor_mul(
                            a_sb[:, i, 0:nw], sg[:, 0:nw], psu[:, 0:nw])

                # ---- down, feature-major: psum[h-tile, tok] ----
                # the very last group is split by columns so the final
                # mul+store chain after the last matmul is half-sized, and
                # its two store descriptor-gens run in parallel on SP + ACT
                last_pi = pi == len(phases) - 1
                for hb in range(HK):
                    split = last_pi and hb == HK - 1
                    cols = [(0, nw // 2), (nw // 2, nw)] if split \
                        else [(0, nw)]
                    for si, (c0, c1) in enumerate(cols):
                        psd = pd.tile([128, TCH], F32, tag="pd")
                        for kc in range(IK):
                            q, j = divmod(kc, 4)
                            nc.tensor.matmul(
                                psd[:, 0 : c1 - c0], wd_t[(e, q)][:, j, hb, :],
                                a_sb[:, kc, c0:c1],
                                start=(kc == 0), stop=(kc == IK - 1))
                        yt = ytpool.tile([128, TCH], BF16, tag="yt")
                        nc.vector.tensor_mul(
                            yt[:, 0 : c1 - c0], psd[:, 0 : c1 - c0],
                            cgb_sb[e][:, w0 + c0 : w0 + c1])
                        # stores ride the ACT HWDGE queue (idle during the
                        # down phase) so they never contend with SP's weight
                        # stream; the very last store goes on SP
                        eng = nc.sync if (split and si == 1) else nc.scalar
                        eng.dma_start(
                            yf_t[e][:, hb, w0 + c0 : w0 + c1],
                            yt[:, 0 : c1 - c0])
    return legalize_waits(nc)


def _bf16(a):
    return np.asarray(a).astype(ml_dtypes.bfloat16)


def _chunk4(w):
    """[2, IK, 128, HK, 128] -> [2, NQ, 128, 4, HK, 128] contiguous."""
    return np.ascontiguousarray(
        w.reshape(2, NQ, 4, 128, HK, 128).transpose(0, 1, 3, 2, 4, 5))


def build_in_maps(x, top_k_indices, norm_weights, mlp_gate, mlp_up, mlp_down, conv_w):
    NT = B * S
    xflat = np.asarray(x, dtype=np.float32).reshape(NT, H)
    xflat_b = _bf16(xflat)
    idxflat = np.asarray(top_k_indices).reshape(NT, KTOP)
    nwflat = np.asarray(norm_weights, dtype=np.float32).reshape(NT, KTOP)

    # combined per-expert coefficients, global
    ce = np.zeros((NT, 4), dtype=np.float32)
    rows = np.arange(NT)
    for k in range(KTOP):
        np.add.at(ce, (rows, idxflat[:, k]), nwflat[:, k])

    # globally balanced routing for the two MLP experts: split every
    # expert's token list evenly across the cores
    lists, Cs = [], []
    for e in range(2):
        glst = np.nonzero(ce[:, e] != 0.0)[0]
        parts = np.array_split(glst, NCORES)
        lists.append(parts)
        Cs.append(max(1, max(len(p) for p in parts)))
    _ROUTE["C"] = Cs
    _ROUTE["lists"] = lists

    # conv experts 2,3 computed fully on the host (see module docstring):
    # depthwise causal conv + silu, weighted by c_e, over each expert's
    # routed tokens only
    cwf = np.asarray(conv_w, dtype=np.float32)            # [2, H, KC]
    conv_add = np.zeros((NT, H), dtype=np.float32)
    for e in range(2):
        glst = np.nonzero(ce[:, 2 + e] != 0.0)[0]
        s_in_seq = glst % S
        z = np.zeros((len(glst), H), dtype=np.float32)
        for j in range(KC):
            src = glst + j - (KC - 1)
            valid = (s_in_seq + j - (KC - 1)) >= 0
            z += np.where(valid[:, None], xflat[src * valid], 0) \
                * cwf[e, :, j][None, :]
        z = z / (1.0 + np.exp(-z))
        conv_add[glst] += z * ce[glst, 2 + e][:, None]
    _ROUTE["conv_add"] = conv_add

    # weights, repacked into 4-i-tile chunks contiguous per partition
    wgr = _chunk4(
        _bf16(mlp_gate).reshape(2, HK, 128, IK, 128).transpose(0, 3, 2, 1, 4))
    wur = _chunk4(
        _bf16(mlp_up).reshape(2, HK, 128, IK, 128).transpose(0, 3, 2, 1, 4))
    wdr = _chunk4(_bf16(mlp_down).reshape(2, IK, 128, HK, 128))

    def fm_pack(cols_bf16, Cx):
        """[n, H] bf16 -> [128, HK*Cx] zero-padded feature-major, packed
        per-window so each window is one contiguous DMA."""
        n = cols_bf16.shape[0]
        arr = np.zeros((H, Cx), dtype=ml_dtypes.bfloat16)
        arr[:, :n] = cols_bf16.T
        a3 = arr.reshape(HK, 128, Cx)
        parts = [
            a3[:, :, w0 : w0 + nw].transpose(1, 0, 2).reshape(128, HK * nw)
            for (w0, nw) in _windows(Cx)
        ]
        return np.ascontiguousarray(np.concatenate(parts, axis=1))

    def bcast_row(vals, Cx):
        v = np.zeros(Cx, dtype=np.float32)
        v[: len(vals)] = vals
        return np.ascontiguousarray(
            np.broadcast_to(v[None, :], (128, Cx))).astype(ml_dtypes.bfloat16)

    in_maps = []
    for i in range(NCORES):
        im = {"wgr": wgr, "wur": wur, "wdr": wdr}
        for e in range(2):
            lst = lists[e][i]
            im[f"xg{e}"] = fm_pack(xflat_b[lst], Cs[e])
            im[f"cgb{e}"] = bcast_row(ce[lst, e], Cs[e])
        in_maps.append(im)
    return in_maps


def assemble(results):
    lists = _ROUTE["lists"]
    out = _ROUTE["conv_add"].copy()
    keys = ["yf0", "yf1"]
    for i, r in enumerate(results):
        for e in range(2):
            lst = lists[e][i]
            n = len(lst)
            yv = np.asarray(r[keys[e]], dtype=np.float32)  # [H, C_e]
            out[lst] += yv[:, :n].T
    return out.reshape(B, S, H)


def kernel(x, top_k_indices, norm_weights, mlp_gate, mlp_up, mlp_down, conv_w):
    in_maps = build_in_maps(
        x, top_k_indices, norm_weights, mlp_gate, mlp_up, mlp_down, conv_w
    )
    nc = build_nc()
    res = run_bass_kernel_spmd(nc, in_maps, core_ids=list(range(NCORES)))
    return assemble(res.results)


# revision 10
# speedup vs baseline: 1.0232x; 1.0037x over previous
"""MixedExpertLayer Trainium2 kernel, v16: device does MLP experts only.

ALL four experts are routed on the host. For the two MLP experts the tokens
with nonzero combined weight c_e = sum_k w_k*[idx_k==e] (~43.75% of tokens)
are gathered into a compacted feature-major stream and split EVENLY across
the 8 cores, so each core processes ~900 tokens per expert.

MLP experts 0,1 (per core, per expert, two windows of <=512 tokens, the
SMALL window first so the critical first transfer is minimal):
  gate/up: psum[i-tile, tok] = sum_h W[h,i]^T x[h, tok]   (feature-major)
  A = silu(g)*u on ACT+DVE
  down (feature-major): psum[h-tile, tok] = sum_i Wd[i,h]^T A[i, tok]
  scaled by c_e via one DVE mul with a host-broadcast coefficient row.

Conv experts 2,3 (depthwise causal conv1d + silu) are computed entirely on
the host: the routed gather already required materializing the tap-shifted
token copies host-side, which costs more than the conv itself.

DMA design (tuned from traces): each dma_start costs ~0.6us serial HWDGE
descriptor-gen plus ~1.5us completion-receipt latency before its semaphore
fires, and transfers on one ring complete FIFO while two active rings split
bandwidth.  So ALL loads ride the SP ring in demand order as few/large
transfers: a single host-packed "head blob" (first gate+up i-tile + the
whole first activation window) unblocks the first matmul group with one
semaphore; gate and up weights are host-interleaved into one tensor and
loaded as half-chunk (1MB) pieces; xg/cgb loads are merged.  Stores ride
the ACT ring (idle during down phases), the final store on SP so the last
two descriptor-gens overlap.

All outputs are compacted feature-major [H, C_e]; the host transposes and
scatter-adds the streams into the zero-initialized result (fp32).
"""

import numpy as np
import ml_dtypes

import concourse.bass as bass
import concourse.mybir as mybir
import concourse.tile as tile
from concourse.bass_utils import run_bass_kernel_spmd

B, S, H, I, KTOP, KC = 4, 4096, 1024, 2048, 2, 4
NCORES = 8
TCH = 512                      # token chunk (matmul N / PSUM bank)
HK = H // 128                  # 8 h-chunks
IK = I // 128                  # 16 i-chunks
NQ = 4                         # i-tile chunks per expert (IK/4)
NWARM = 14                     # PE clock-warmup matmuls
BF16 = mybir.dt.bfloat16
F32 = mybir.dt.float32
AF = mybir.ActivationFunctionType

# routing state set by build_in_maps: per-expert device capacities and
# per-(expert, core) global token lists
_ROUTE = {"C": [1024, 1024], "lists": None}


def legalize_waits(nc):
    """This walrus build encodes exactly one sync-wait per instruction
    (single NEURON_ISA_TPB_EVENTS slot); Tile emits up to 3 plus a multi-wait
    tail Drain. Split extra waits onto wait-only EventSemaphore carriers
    inserted immediately before the instruction (same engine, same position,
    so no reordering and no deadlock risk)."""
    f = nc.m.functions[0]
    for blk in f.blocks:
        new = []
        for ins in list(blk.instructions):
            si = ins.sync_info
            if si is not None and si.on_wait and len(si.on_wait) > 1:
                best, order = {}, []
                for w in si.on_wait:
                    k = (w.sync_type, w.id, w.wait_mode)
                    if k not in best:
                        best[k] = w
                        order.append(k)
                    elif (w.wait_value or 0) > (best[k].wait_value or 0):
                        best[k] = w
                waits = [best[k] for k in order]
                for j, w in enumerate(waits[:-1]):
                    ev = mybir.InstEventSemaphore(
                        name=f"{ins.name}-lw{j}", engine=ins.engine, ins=[], outs=[],
                    )
                    ev.sync_info = mybir.SyncInfo(on_wait=[w], on_update=[])
                    new.append(ev)
                si.on_wait = [waits[-1]]
                ins.sync_info = si
            new.append(ins)
        blk.instructions = new
    return nc


def _windows(Ce):
    """Windows (w0, n) covering Ce tokens, <=TCH each, SMALLEST first."""
    if Ce <= TCH:
        return [(0, Ce)]
    return [(0, Ce - TCH), (Ce - TCH, TCH)]


def build_nc():
    C0, C1 = _ROUTE["C"][:2]
    wins = [_windows(C0), _windows(C1)]
    n00 = wins[0][0][1]
    nc = bass.Bass(num_devices=NCORES)
    # head blob: first gate+up i-tile (2*HK*128 per partition) followed by
    # expert 0's first activation window - one DMA, one semaphore
    headr = nc.dram_tensor("headr", [128, 2048 + HK * n00], BF16,
                           kind="ExternalInput")
    # activations packed per-(expert, window): [128, sum_w HK*nw]
    xg0 = nc.dram_tensor("xg0", [128, HK * C0], BF16, kind="ExternalInput")
    xg1 = nc.dram_tensor("xg1", [128, HK * C1], BF16, kind="ExternalInput")
    # gate+up interleaved, 4-i-tile chunks: [e, q, part, j, {g,u}, hk, 128]
    wgur = nc.dram_tensor("wgur", [2, NQ, 128, 4, 2, HK, 128], BF16,
                          kind="ExternalInput")
    wdr = nc.dram_tensor("wdr", [2, NQ, 128, 4, HK, 128], BF16,
                         kind="ExternalInput")
    cgbr = nc.dram_tensor("cgbr", [128, C0 + C1], BF16, kind="ExternalInput")
    yf0 = nc.dram_tensor("yf0", [H, C0], BF16, kind="ExternalOutput")
    yf1 = nc.dram_tensor("yf1", [H, C1], BF16, kind="ExternalOutput")

    yf_t = [y.rearrange("(o p) t -> p o t", p=128) for y in (yf0, yf1)]
    xg_d = [xg0, xg1]
    phases = [(0, wi, w0, n) for wi, (w0, n) in enumerate(wins[0])] + \
             [(1, wi, w0, n) for wi, (w0, n) in enumerate(wins[1])]

    with tile.TileContext(nc) as tc:
        with (
            tc.tile_pool(name="singles", bufs=1) as singles,
            tc.tile_pool(name="wpool", bufs=4) as wpool,
            tc.tile_pool(name="wdpool", bufs=4) as wdpool,
            tc.tile_pool(name="apool", bufs=2) as apool,
            tc.tile_pool(name="sgp", bufs=2) as sgpool,
            tc.tile_pool(name="ytp", bufs=3) as ytpool,
            tc.tile_pool(name="psg", bufs=3, space="PSUM") as psgp,
            tc.tile_pool(name="psu", bufs=3, space="PSUM") as psup,
            tc.tile_pool(name="pd", bufs=2, space="PSUM") as pd,
        ):
            # ---- PE clock warmup: dummy matmuls with no DMA deps run while
            # the first input tiles stream in, so the HAM clock-gate ramp
            # (1.2 -> 2.4GHz after ~3.4-5us of sustained PE activity)
            # finishes about when the real matmuls start ----
            scr = singles.tile([128, TCH], BF16)
            nc.gpsimd.memset(scr, 0.0)
            pw = pd.tile([128, TCH], F32, tag="pd", name="pwarm")
            for r in range(NWARM):
                nc.tensor.matmul(pw, scr[:, 0:128], scr,
                                 start=(r == 0), stop=(r == NWARM - 1))

            head_sb = singles.tile([128, 2048 + HK * n00], BF16)
            # windows beyond (0,0) get their own tiles; (0,0) lives in head
            xgw_sb = {
                (e, wi): singles.tile([128, HK * n], BF16, name=f"xg{e}w{wi}")
                for e in range(2)
                for wi, (w0, n) in enumerate(wins[e]) if (e, wi) != (0, 0)
            }
            cgb_sb = singles.tile([128, C0 + C1], BF16, name="cgb")

            def xgw_view(e, wi):
                return head_sb[:, 2048:] if (e, wi) == (0, 0) \
                    else xgw_sb[(e, wi)]

            def xg_load(e, wi):
                off = HK * wins[e][wi][0]
                n = wins[e][wi][1]
                nc.sync.dma_start(xgw_sb[(e, wi)],
                                  xg_d[e][:, off : off + HK * n])

            wgu_t = {}  # (e, q) -> sbuf chunk tile [128, 4, 2, HK, 128]
            wd_t = {}

            def wgu_alloc(e, q):
                wgu_t[(e, q)] = wpool.tile([128, 4, 2, HK, 128], BF16,
                                           tag="wgu", name="wgut")

            def wgu_load(e, q, j0, j1):
                nc.sync.dma_start(wgu_t[(e, q)][:, j0:j1],
                                  wgur[e, q, :, j0:j1])

            def lhs_gu(e, q, j, gu, kc):
                if (e, q, j) == (0, 0, 0):
                    o = gu * 1024 + kc * 128
                    return head_sb[:, o : o + 128]
                return wgu_t[(e, q)][:, j, gu, kc, :]

            def wd_load(e, q):
                wd_t[(e, q)] = wdpool.tile([128, 4, HK, 128], BF16,
                                           tag="wd", name="wdt")
                nc.sync.dma_start(wd_t[(e, q)], wdr[e, q])

            # head: the blob (one DMA, one semaphore unblocks group 0),
            # then the rest of chunk 0
            nc.sync.dma_start(head_sb, headr[:])
            wgu_alloc(0, 0)
            wgu_load(0, 0, 1, 2)
            wgu_load(0, 0, 2, 4)

            # deferred SP loads at (q, j) points of expert 0's first window,
            # in demand order (ring FIFO makes issue order completion order)
            def defer_e0(q, j):
                if (q, j) == (0, 1):
                    wgu_alloc(0, 1); wgu_load(0, 1, 0, 2)
                elif (q, j) == (0, 2):
                    wgu_load(0, 1, 2, 4)
                elif (q, j) == (1, 0):
                    wgu_alloc(0, 2); wgu_load(0, 2, 0, 2)
                elif (q, j) == (1, 1):
                    wgu_load(0, 2, 2, 4)
                elif (q, j) == (1, 2):
                    wgu_alloc(0, 3); wgu_load(0, 3, 0, 2)
                elif (q, j) == (1, 3):
                    wgu_load(0, 3, 2, 4)
                elif (q, j) == (2, 0):
                    wd_load(0, 0)
                elif (q, j) == (2, 1):
                    wd_load(0, 1)
                elif (q, j) == (2, 2):
                    wd_load(0, 2)
                elif (q, j) == (2, 3):
                    wd_load(0, 3)
                elif (q, j) == (3, 0):
                    nc.sync.dma_start(cgb_sb, cgbr[:])
                elif (q, j) == (3, 1):
                    if len(wins[0]) > 1:
                        xg_load(0, 1)
                elif (q, j) == (3, 2):
                    xg_load(1, 0)
                elif (q, j) == (3, 3):
                    if len(wins[1]) > 1:
                        xg_load(1, 1)

            def defer_e1(q, j):
                if j == 0:
                    wgu_alloc(1, q); wgu_load(1, q, 0, 2)
                elif j == 1:
                    wgu_load(1, q, 2, 4)
                elif q >= 2 and j >= 2:
                    wd_load(1, 2 * (q - 2) + (j - 2))

            for pi, (e, wi, w0, nw) in enumerate(phases):
                xgw = xgw_view(e, wi)
                wload = (wi == 0)
                ceb = cgb_sb[:, (0 if e == 0 else C0) + w0 :
                             (0 if e == 0 else C0) + w0 + nw]
                # ---- gate/up -> A (feature-major [I, nw]) ----
                a_sb = apool.tile([128, IK, TCH], BF16, tag="a")
                for q in range(NQ):
                    for j in range(4):
                        if wload and not (pi == 0 and q == 0 and j == 0):
                            (defer_e0 if e == 0 else defer_e1)(q, j)
                        i = 4 * q + j
                        psg = psgp.tile([128, TCH], F32, tag="pg")
                        psu = psup.tile([128, TCH], F32, tag="pu")
                        for kc in range(HK):
                            nc.tensor.matmul(
                                psg[:, 0:nw], lhs_gu(e, q, j, 0, kc),
                                xgw[:, kc * nw : (kc + 1) * nw],
                                start=(kc == 0), stop=(kc == HK - 1))
                        for kc in range(HK):
                            nc.tensor.matmul(
                                psu[:, 0:nw], lhs_gu(e, q, j, 1, kc),
                                xgw[:, kc * nw : (kc + 1) * nw],
                                start=(kc == 0), stop=(kc == HK - 1))
                        sg = sgpool.tile([128, TCH], F32, tag="sg")
                        nc.scalar.activation(
                            out=sg[:, 0:nw], in_=psg[:, 0:nw], func=AF.Silu)
                        nc.vector.tensor_mul(
                            a_sb[:, i, 0:nw], sg[:, 0:nw], psu[:, 0:nw])

                # ---- down, feature-major: psum[h-tile, tok] ----
                # the very last group is split by columns so the final
                # mul+store chain after the last matmul is half-sized
                last_pi = pi == len(phases) - 1
                for hb in range(HK):
                    split = last_pi and hb == HK - 1
                    cols = [(0, nw // 2), (nw // 2, nw)] if split \
                        else [(0, nw)]
                    for si, (c0, c1) in enumerate(cols):
                        psd = pd.tile([128, TCH], F32, tag="pd")
                        for kc in range(IK):
                            q, j = divmod(kc, 4)
                            nc.tensor.matmul(
                                psd[:, 0 : c1 - c0], wd_t[(e, q)][:, j, hb, :],
                                a_sb[:, kc, c0:c1],
                                start=(kc == 0), stop=(kc == IK - 1))
                        yt = ytpool.tile([128, TCH], BF16, tag="yt")
                        nc.vector.tensor_mul(
                            yt[:, 0 : c1 - c0], psd[:, 0 : c1 - c0],
                            ceb[:, c0:c1])
                        # stores ride the ACT ring (idle during the down
                        # phase); the very last store goes on SP so the
                        # final two descriptor-gens run in parallel
                        eng = nc.sync if (split and si == 1) else nc.scalar
                        eng.dma_start(
                            yf_t[e][:, hb, w0 + c0 : w0 + c1],
                            yt[:, 0 : c1 - c0])
    return legalize_waits(nc)


def _bf16(a):
    return np.asarray(a).astype(ml_dtypes.bfloat16)


def build_in_maps(x, top_k_indices, norm_weights, mlp_gate, mlp_up, mlp_down, conv_w):
    NT = B * S
    xflat = np.asarray(x, dtype=np.float32).reshape(NT, H)
    xflat_b = _bf16(xflat)
    idxflat = np.asarray(top_k_indices).reshape(NT, KTOP)
    nwflat = np.asarray(norm_weights, dtype=np.float32).reshape(NT, KTOP)

    # combined per-expert coefficients, global
    ce = np.zeros((NT, 4), dtype=np.float32)
    rows = np.arange(NT)
    for k in range(KTOP):
        np.add.at(ce, (rows, idxflat[:, k]), nwflat[:, k])

    # globally balanced routing for the two MLP experts: split every
    # expert's token list evenly across the cores
    lists, Cs = [], []
    for e in range(2):
        glst = np.nonzero(ce[:, e] != 0.0)[0]
        parts = np.array_split(glst, NCORES)
        lists.append(parts)
        Cs.append(max(1, max(len(p) for p in parts)))
    _ROUTE["C"] = Cs
    _ROUTE["lists"] = lists

    # conv experts 2,3 computed fully on the host (see module docstring):
    # depthwise causal conv + silu, weighted by c_e, over each expert's
    # routed tokens only
    cwf = np.asarray(conv_w, dtype=np.float32)            # [2, H, KC]
    conv_add = np.zeros((NT, H), dtype=np.float32)
    for e in range(2):
        glst = np.nonzero(ce[:, 2 + e] != 0.0)[0]
        s_in_seq = glst % S
        z = np.zeros((len(glst), H), dtype=np.float32)
        for j in range(KC):
            src = glst + j - (KC - 1)
            valid = (s_in_seq + j - (KC - 1)) >= 0
            z += np.where(valid[:, None], xflat[src * valid], 0) \
                * cwf[e, :, j][None, :]
        z = z / (1.0 + np.exp(-z))
        conv_add[glst] += z * ce[glst, 2 + e][:, None]
    _ROUTE["conv_add"] = conv_add

    # gate+up interleaved, repacked into 4-i-tile chunks per partition
    wg_ = _bf16(mlp_gate).reshape(2, HK, 128, IK, 128).transpose(0, 3, 2, 1, 4)
    wu_ = _bf16(mlp_up).reshape(2, HK, 128, IK, 128).transpose(0, 3, 2, 1, 4)
    wgu6 = np.stack([wg_, wu_], axis=3)          # [2, IK, 128, 2, HK, 128]
    wgur = np.ascontiguousarray(
        wgu6.reshape(2, NQ, 4, 128, 2, HK, 128).transpose(0, 1, 3, 2, 4, 5, 6))
    wdr = np.ascontiguousarray(
        _bf16(mlp_down).reshape(2, IK, 128, HK, 128)
        .reshape(2, NQ, 4, 128, HK, 128).transpose(0, 1, 3, 2, 4, 5))
    wgu_head = np.ascontiguousarray(wgur[0, 0, :, 0].reshape(128, 2048))

    def fm_pack(cols_bf16, Cx):
        """[n, H] bf16 -> [128, HK*Cx] zero-padded feature-major, packed
        per-window so each window is one contiguous DMA."""
        n = cols_bf16.shape[0]
        arr = np.zeros((H, Cx), dtype=ml_dtypes.bfloat16)
        arr[:, :n] = cols_bf16.T
        a3 = arr.reshape(HK, 128, Cx)
        parts = [
            a3[:, :, w0 : w0 + nw].transpose(1, 0, 2).reshape(128, HK * nw)
            for (w0, nw) in _windows(Cx)
        ]
        return np.ascontiguousarray(np.concatenate(parts, axis=1))

    def bcast_row(vals, Cx):
        v = np.zeros(Cx, dtype=np.float32)
        v[: len(vals)] = vals
        return np.broadcast_to(v[None, :], (128, Cx))

    n00 = _windows(Cs[0])[0][1]
    in_maps = []
    for i in range(NCORES):
        im = {"wgur": wgur, "wdr": wdr}
        xgs = [fm_pack(xflat_b[lists[e][i]], Cs[e]) for e in range(2)]
        im["xg0"], im["xg1"] = xgs
        im["headr"] = np.ascontiguousarray(
            np.concatenate([wgu_head, xgs[0][:, : HK * n00]], axis=1))
        im["cgbr"] = np.ascontiguousarray(np.concatenate(
            [bcast_row(ce[lists[e][i], e], Cs[e]) for e in range(2)],
            axis=1)).astype(ml_dtypes.bfloat16)
        in_maps.append(im)
    return in_maps


def assemble(results):
    lists = _ROUTE["lists"]
    out = _ROUTE["conv_add"].copy()
    keys = ["yf0", "yf1"]
    for i, r in enumerate(results):
        for e in range(2):
            lst = lists[e][i]
            n = len(lst)
            yv = np.asarray(r[keys[e]], dtype=np.float32)  # [H, C_e]
            out[lst] += yv[:, :n].T
    return out.reshape(B, S, H)


def kernel(x, top_k_indices, norm_weights, mlp_gate, mlp_up, mlp_down, conv_w):
    in_maps = build_in_maps(
        x, top_k_indices, norm_weights, mlp_gate, mlp_up, mlp_down, conv_w
    )
    nc = build_nc()
    res = run_bass_kernel_spmd(nc, in_maps, core_ids=list(range(NCORES)))
    return assemble(res.results)
